# revision 82
# baseline (speedup 1.0000x reference)
"""Multi-head attention layer on 8 Trainium2 NeuronCores.

Problem: B=4, S=2048, D=1024, H=16 heads (DH=64), int mask over keys.
Sharding: core c -> batch b=c//2, head-group hg=c%2 (8 heads each).
Each core computes its heads' full S x S attention independently; no
collectives.  Host-side prep folds everything expensive into the input
layout:

  - X is passed TRANSPOSED ([D, S], d on partitions) so Q^T/K^T come out
    of the projection matmuls directly in the [d, S] layout the attention
    matmuls consume (zero on-device transposes).
  - A ones-row is appended to X^T and the bias row to W so biases ride the
    projection contraction; padded with zeros to a whole 128-row K-tile so
    every projection matmul stays in the PE's 128x128 tiling mode.
  - Wk is pre-scaled by 1/sqrt(DH) on the host.
  - The additive key mask is passed as a [128, S] tensor (rows 0 and 64
    hold -10000*(1-mask), the rest zeros) and added to the scores inside
    the PE accumulation group as a zero-padded K=64 matmul, so the exp on
    ScalarE needs no per-partition bias and can span two PSUM banks.
  - V gets a ones-column (65th output row of the PV matmul) so the softmax
    denominator accumulates for free alongside the numerator.

Attention runs entirely in the PE's 64-row tiling mode: head pairs are
packed onto array halves T0 (SBUF partitions 0-63) and T8 (64-127) and run
concurrently; the K=128 PV contraction is split into T0/T8 halves that
accumulate in separate PSUM banks and are summed on the DVE at the end.
"""

import os
import sys

import numpy as np
import ml_dtypes

for _p in ("/opt/trn_rl_repo", "/opt/pypackages"):
    if os.path.isdir(_p) and _p not in sys.path:
        sys.path.append(_p)

import concourse.bass as bass
import concourse.mybir as mybir
import concourse.tile as tile
from concourse.tile import add_dep_helper
from contextlib import ExitStack

BF16 = mybir.dt.bfloat16
F32 = mybir.dt.float32

B, S, D, H, DH = 4, 2048, 1024, 16, 64
NCORES = 8
DCOL = 512          # head-group width (8 heads x 64)
NM = 4              # 128-wide dcol tiles of the head group
NQC = 4             # 512-wide query chunks
EXPFN = mybir.ActivationFunctionType.Exp


def _chunks(total, size=512):
    out, o = [], 0
    while o < total:
        c = min(size, total - o)
        out.append(slice(o, o + c))
        o += c
    return out


def build_nc(nk: int, skv: int) -> bass.Bass:
    """nk: K-tiles over the hidden dim (8, or 9 with a bias row).
    skv: compacted+padded key/value sequence length (multiple of 128).
    Masked-out keys are compacted away on the host (their exp() is an
    exact 0 in the reference); pad keys carry a -10000 mask bias."""
    NK = nk
    KPAD = NK * 128
    NJ = skv // 128     # key tiles for scores / PV / V-projection
    nc = bass.Bass()
    xt_d = nc.declare_dram_parameter("xt", [KPAD, S], BF16, isOutput=False)
    xkv_d = nc.declare_dram_parameter("xkv", [KPAD, skv], BF16,
                                      isOutput=False)
    wq_d = nc.declare_dram_parameter("wq", [KPAD, DCOL], BF16, isOutput=False)
    wk_d = nc.declare_dram_parameter("wk", [KPAD, DCOL], BF16, isOutput=False)
    wv_d = nc.declare_dram_parameter("wv", [KPAD, DCOL], BF16, isOutput=False)
    mz_d = nc.declare_dram_parameter("mz", [128, skv], BF16, isOutput=False)
    out_d = nc.declare_dram_parameter("out", [DCOL, S], F32, isOutput=True)

    with tile.TileContext(nc) as tc, ExitStack() as ctx:
        const = ctx.enter_context(tc.tile_pool(name="const", bufs=1))
        spool = ctx.enter_context(tc.tile_pool(name="sc", bufs=2, space="PSUM"))
        pvpool = ctx.enter_context(tc.tile_pool(name="pv", bufs=1, space="PSUM"))
        expool = ctx.enter_context(tc.tile_pool(name="ex", bufs=10))
        comb = ctx.enter_context(tc.tile_pool(name="comb", bufs=4))
        outp = ctx.enter_context(tc.tile_pool(name="outp", bufs=5))

        # ---------------- persistent SBUF tensors ----------------
        xt = [const.tile([128, S], BF16, tag=f"xt{k}", name=f"xt{k}")
              for k in range(NK)]
        xkv = [const.tile([128, skv], BF16, tag=f"xkv{k}", name=f"xkv{k}")
               for k in range(NK)]
        wq = [const.tile([128, DCOL], BF16, tag=f"wq{k}", name=f"wq{k}")
              for k in range(NK)]
        wk = [const.tile([128, DCOL], BF16, tag=f"wk{k}", name=f"wk{k}")
              for k in range(NK)]
        wv = [const.tile([128, DCOL], BF16, tag=f"wv{k}", name=f"wv{k}")
              for k in range(NK)]
        mz = const.tile([128, skv], BF16, tag="mz")
        ones = const.tile([128, DCOL], BF16, tag="ones")
        qt = const.tile([128, NM, S], BF16, tag="qt")    # Q^T  [dcol, S]
        kt = const.tile([128, NM, skv], BF16, tag="kt")  # K^T (pre-scaled)
        # V (cols 0-63) + 64 ones-columns (64-127) per (key tile, head): the
        # PV matmul then emits the numerator on partitions 0-63 AND the
        # softmax denominator replicated across partitions 64-127 — the
        # partition broadcast of 1/den is never needed.
        vo = const.tile([128, NJ, 8, 128], BF16, tag="vo")

        for k in range(NK):
            nc.sync.dma_start(out=xkv[k],
                              in_=xkv_d[k * 128:(k + 1) * 128, :])
            nc.sync.dma_start(out=wk[k], in_=wk_d[k * 128:(k + 1) * 128, :])
            nc.sync.dma_start(out=wv[k], in_=wv_d[k * 128:(k + 1) * 128, :])
        nc.sync.dma_start(out=mz, in_=mz_d[:, :])
        for k in range(NK):
            nc.sync.dma_start(out=xt[k], in_=xt_d[k * 128:(k + 1) * 128, :])
            nc.sync.dma_start(out=wq[k], in_=wq_d[k * 128:(k + 1) * 128, :])
        nc.gpsimd.memset(ones, 1.0)
        ms_pool = nc.gpsimd.memset(vo, 1.0)  # ones cols; V copies fill 0-63
        # DVE pre-touch: observe the gpsimd memset tick once, so the per-tile
        # V copies don't each need a second (Pool) sync wait — the DVE ISA
        # struct has a single wait slot.
        nc.vector.memset(vo[0:1, 0, 0, 64:65], 1.0)

        # ---------------- QKV projections (128x128 PE mode) ----------------
        # K^T: out[m, s] = sum_k W[k, m] * Xkv^T[k, s]
        def emit_proj(wtiles, dest, src, csl):
            for m in range(NM):
                ps = spool.tile([128, 1024], F32, tag="sc", name=f"pj{m}")
                pslice = ps[:, 0:csl.stop - csl.start]
                for k in range(NK):
                    nc.tensor.matmul(
                        pslice,
                        lhsT=wtiles[k][:, m * 128:(m + 1) * 128],
                        rhs=src[k][:, csl],
                        start=(k == 0),
                        stop=(k == NK - 1),
                    )
                nc.vector.tensor_copy(dest[:, m, csl], pslice)

        def emit_proj_one(wtiles, dest, src, m, csl):
            ps = spool.tile([128, 1024], F32, tag="sc",
                            name=f"pj{m}_{csl.start}")
            pslice = ps[:, 0:csl.stop - csl.start]
            for k in range(NK):
                nc.tensor.matmul(
                    pslice,
                    lhsT=wtiles[k][:, m * 128:(m + 1) * 128],
                    rhs=src[k][:, csl],
                    start=(k == 0),
                    stop=(k == NK - 1),
                )
            nc.vector.tensor_copy(dest[:, m, csl], pslice)

        def emit_proj64(wtiles, dest, src, m, csl):
            """Projection group in the PE's 64-row tiling mode: the K=128
            contraction is split across array halves T0/T8 into two PSUM
            banks, merged by one DVE add.  Same PE wall time as 128-mode,
            but NO tiling-mode switch — so these groups can interleave
            with the (64-mode) attention stream penalty-free."""
            n = csl.stop - csl.start
            ps = spool.tile([128, 1024], F32, tag="sc",
                            name=f"pj64_{m}_{csl.start}")
            for k in range(NK):
                kw = dict(start=(k == 0), stop=(k == NK - 1))
                nc.tensor.matmul(
                    ps[:, 0:n],
                    lhsT=wtiles[k][0:64, m * 128:(m + 1) * 128],
                    rhs=src[k][0:64, csl], **kw)
                nc.tensor.matmul(
                    ps[:, 512:512 + n],
                    lhsT=wtiles[k][64:128, m * 128:(m + 1) * 128],
                    rhs=src[k][64:128, csl], **kw)
            half = comb.tile([128, 512], F32, tag="pjh",
                             name=f"pjh{m}_{csl.start}")
            nc.vector.tensor_copy(half[:, 0:n], ps[:, 512:512 + n])
            nc.vector.tensor_add(dest[:, m, csl], ps[:, 0:n], half[:, 0:n])

        # Upfront (128-mode, before the one switch into attention): only
        # what the first attention iterations consume.  The m2/m3 K^T/Q^T
        # groups are injected into the attention tick stream below in
        # 64-row mode, riding the PE slack under the ScalarE exp pace.
        for csl in _chunks(skv):
            emit_proj_one(wk, kt, xkv, 0, csl)
            emit_proj_one(wk, kt, xkv, 1, csl)
        # V: out[s, c] = sum_k Xkv^T[k, s] * Wv[k, c], strided into vo
        for st in range(NJ):
            ps = spool.tile([128, 1024], F32, tag="sc")
            pslice = ps[:, 0:512]
            for k in range(NK):
                nc.tensor.matmul(
                    pslice,
                    lhsT=xkv[k][:, st * 128:(st + 1) * 128],
                    rhs=wv[k],
                    start=(k == 0),
                    stop=(k == NK - 1),
                )
            nc.vector.tensor_copy(
                vo[:, st, :, 0:64],
                pslice.rearrange("p (h d) -> p h d", h=8),
            )
        # Q^T m0/m1: its input DMAs overlap the K/V compute
        for csl in _chunks(S):
            emit_proj_one(wq, qt, xt, 0, csl)
            emit_proj_one(wq, qt, xt, 1, csl)
        inject = []
        for m in (2, 3):
            for csl in _chunks(skv):
                inject.append(
                    lambda m=m, csl=csl: emit_proj64(wk, kt, xkv, m, csl))
            for csl in _chunks(S):
                inject.append(
                    lambda m=m, csl=csl: emit_proj64(wq, qt, xt, m, csl))
        # one group per 6 ticks matches the PE slack under the exp pace;
        # m2 finishes well before iteration 8 (tick 80), m3 before 120
        inject_at = {6 * i: fn for i, fn in enumerate(inject)}

        # ------- attention: software-pipelined global instruction stream --
        # Engines execute their instruction streams strictly in order, so a
        # single instruction parked on an unmet semaphore stalls everything
        # traced after it on that engine.  To keep the ScalarE exp stream
        # (the throughput limit) dense, the trace is emitted as one global
        # pipeline: scores/exp for tick g, PV matmuls for tick g-LAG, and
        # each iteration's softmax-normalization tail staggered a few ticks
        # after its last PV so all its waits are satisfied on arrival.
        LAG = 8
        NIT = NM * NQC          # 16 (pair, qchunk) iterations
        NG = NIT * NJ           # 256 scores/exp ticks
        tail_deps = []
        lasts = {}
        ex_ring = {}            # tick -> exp tile
        pv_of = {}              # it -> [pva0, pva1, pvb0, pvb1]
        cstate = {}             # (it, hh) -> dict with combine intermediates
        pending = {}            # tick -> list of closures
        exp_of_it = {}          # it -> first exp instruction (NOP anchors)

        def emit_scores(g):
            it, j = divmod(g, NJ)
            p, q = divmod(it, NQC)
            qsl = slice(q * 512, (q + 1) * 512)
            jsl = slice(j * 128, (j + 1) * 128)
            ps = spool.tile([128, 1024], F32, tag="sc", name=f"ps{g}")
            # scores^T + mask, head A on array half T0 (partitions 0-63)
            nc.tensor.matmul(
                ps[:, 0:512], lhsT=kt[0:64, p, jsl],
                rhs=qt[0:64, p, qsl], start=True, stop=False)
            nc.tensor.matmul(
                ps[:, 0:512], lhsT=mz[0:64, jsl],
                rhs=ones[0:64, :], start=False, stop=True)
            # head B on T8 (partitions 64-127)
            nc.tensor.matmul(
                ps[:, 512:1024], lhsT=kt[64:128, p, jsl],
                rhs=qt[64:128, p, qsl], start=True, stop=False)
            nc.tensor.matmul(
                ps[:, 512:1024], lhsT=mz[64:128, jsl],
                rhs=ones[64:128, :], start=False, stop=True)
            # exp over both heads' scores in one ScalarE pass
            ex = expool.tile([128, 1024], BF16, tag="ex", name=f"ex{g}")
            lasts["exp"] = nc.scalar.activation(ex, ps, EXPFN)
            ex_ring[g] = ex

        def emit_pv(g):
            it, j = divmod(g, NJ)
            p, q = divmod(it, NQC)
            if j == 0:
                pv_of[it] = [
                    pvpool.tile([128, 512], F32, tag=t, name=f"{t}_{it}")
                    for t in ("pva0", "pva1", "pvb0", "pvb1")]
            pva0, pva1, pvb0, pvb1 = pv_of[it]
            ex = ex_ring.pop(g)
            kw = dict(start=(j == 0), stop=(j == NJ - 1))
            nc.tensor.matmul(pva0, lhsT=vo[0:64, j, 2 * p, :],
                             rhs=ex[0:64, 0:512], **kw)
            nc.tensor.matmul(pva1, lhsT=vo[64:128, j, 2 * p, :],
                             rhs=ex[64:128, 0:512], **kw)
            nc.tensor.matmul(pvb0, lhsT=vo[0:64, j, 2 * p + 1, :],
                             rhs=ex[0:64, 512:1024], **kw)
            nc.tensor.matmul(pvb1, lhsT=vo[64:128, j, 2 * p + 1, :],
                             rhs=ex[64:128, 512:1024], **kw)
            if j == NJ - 1:
                # Stage the tail so every PSUM-slot-freeing copy/add runs
                # BEFORE the (slow, 3.3us) reciprocals: the next iteration's
                # PV matmuls wait on these slot releases, and a reciprocal
                # queued in between would stall the whole PE stream.
                pending.setdefault(g + 1, []).append(
                    lambda it=it: (combine1(it, 0), combine1(it, 1)))
                for hh in (0, 1):
                    pending.setdefault(g + 2 + hh, []).append(
                        lambda it=it, hh=hh: combine2(it, hh))
                    pending.setdefault(g + 4 + hh, []).append(
                        lambda it=it, hh=hh: combine3(it, hh))

        def combine1(it, hh):
            """Merge the two PV half-sums (frees both pv PSUM banks)."""
            pv0, pv1 = pv_of[it][2 * hh], pv_of[it][2 * hh + 1]
            s = cstate[(it, hh)] = {}
            tmp = comb.tile([128, 512], F32, tag="tmp", name=f"tmp{it}_{hh}")
            nc.vector.tensor_copy(tmp, pv1)
            ssum = comb.tile([128, 512], F32, tag="ssum",
                             name=f"ssum{it}_{hh}")
            s["add"] = nc.vector.tensor_add(ssum, pv0, tmp)
            s["ssum"] = ssum

        def combine2(it, hh):
            """1/den computed IN PLACE at base partition 64 (no cross-base),
            then DMA-shifted down to partitions 0-63 while the next head's
            reciprocal keeps the DVE busy — the transfer never stalls DVE."""
            s = cstate[(it, hh)]
            rect = comb.tile([128, 512], F32, tag="rect",
                             name=f"rect{it}_{hh}")
            rc_i = nc.vector.reciprocal(rect[64:128, :], s["ssum"][64:128, :])
            rec = comb.tile([64, 512], F32, tag="rec", name=f"rec{it}_{hh}")
            # SP NOP dep'd on the DMA's data producer absorbs the DVE wait
            # into the SP observed clock (1-wait DMA ISA struct)
            nop_i = nc.sync.nop(nofuse=True, hint=f"dshw{it}_{hh}")
            add_dep_helper(nop_i.ins, rc_i.ins, reason="dsh wait carry")
            s["dma"] = nc.sync.dma_start(out=rec, in_=rect[64:128, :])
            s["rec"] = rec

        def combine3(it, hh):
            """Final multiply on the otherwise-idle GpSimd engine (all
            operands are SBUF) — keeps the rec-DMA completion wait and the
            multiply itself off the busy DVE stream."""
            p, q = divmod(it, NQC)
            s = cstate.pop((it, hh))
            ot = outp.tile([64, 512], F32, tag="ot", name=f"ot{it}_{hh}")
            # absorb the rec-DMA and ssum-producer waits into the GpSimd
            # observed clock (1-wait engine ISA structs)
            scr = comb.tile([1, 1], F32, tag="scr", name=f"scr{it}_{hh}")
            m1 = nc.gpsimd.memset(scr, 0.0)
            add_dep_helper(m1.ins, s["dma"].ins, reason="rec wait carry")
            m2 = nc.gpsimd.memset(scr, 0.0)
            add_dep_helper(m2.ins, s["add"].ins, reason="ssum wait carry")
            nc.gpsimd.memset(ot[0:1, 0:1], 0.0)
            lasts["mul"] = nc.gpsimd.tensor_mul(
                ot, s["ssum"][0:64, :], s["rec"])
            nop_i = nc.sync.nop(nofuse=True, hint=f"stw{it}_{hh}")
            add_dep_helper(nop_i.ins, lasts["mul"].ins,
                           reason="store wait carry")
            row0 = p * 128 + hh * 64
            st_i = nc.sync.dma_start(
                out=out_d[row0:row0 + 64, q * 512:(q + 1) * 512], in_=ot)
            tail_deps.append(st_i)

        for g in range(NG + LAG + 8):
            for fn in pending.pop(g, ()):
                fn()
            if g < NG:
                emit_scores(g)
                if g % NJ == 0:
                    exp_of_it[g // NJ] = lasts["exp"]
            if g % 5 == 0:
                # Spread zero-wait SP slots through the stream for the
                # wait legalizer.  The dep is only for PLACEMENT: use a
                # long-completed instruction (two iterations back) so
                # the NOP's wait never stalls the SP stream — a wait on
                # the current exp would hold up every store/DMA queued
                # behind it (convoy through the ot-slot WAR).
                anchor = exp_of_it.get(min(g // NJ, NIT - 1) - 1, ms_pool)
                for k in range(8):
                    nop_i = nc.sync.nop(nofuse=True, hint=f"pad{g}_{k}")
                    add_dep_helper(nop_i.ins, anchor.ins,
                                   reason="legalizer slot padding")
            if LAG <= g < NG + LAG:
                emit_pv(g - LAG)
            fn = inject_at.pop(g, None)
            if fn is not None:
                fn()
        for g in sorted(pending):
            for fn in pending[g]:
                fn()
        pending.clear()
        # Trailing SP no-ops, each depending on one late instruction: the
        # SP sequencer then observes every proc's final semaphore tick
        # before the kernel-tail Drain, whose ISA struct takes only a
        # single sync wait, so Tile elides the rest.
        last_store = tail_deps[-1]
        tail_deps += [lasts["exp"], lasts["mul"], ms_pool]
        for d in tail_deps:
            nop_i = nc.sync.nop(nofuse=True, hint="tailpad")
            add_dep_helper(nop_i.ins, d.ins,
                           reason="spread tail drain waits")
        for _ in range(10):  # zero-wait late slots for the legalizer
            nop_i = nc.sync.nop(nofuse=True, hint="tailpad2")
            add_dep_helper(nop_i.ins, last_store.ins,
                           reason="late zero-wait slot")
    _spread_matmul_waits(nc)
    return nc


def _spread_matmul_waits(nc):
    """The walrus in this container accepts only ONE sync-wait command per
    compute-engine ISA struct (Matmult/Activation/TensorCopy/...), but the
    Tile scheduler sometimes attaches two.  Fix: move excess waits onto an
    earlier instruction of the same engine (which executes first, so the
    ordering the wait enforces is preserved).

    Safety: a wait (sem, v) may move to predecessor p only if the
    instruction whose update makes sem reach v is scheduled BEFORE p.
    That keeps every wait's producer strictly earlier in the schedule, so
    the event order stays acyclic (no introduced deadlocks)."""
    import bass_rust

    SKIP_OPCODES = {"EventSemaphore"}
    if True:
        insts = [i for blk in nc.m.functions[0].blocks
                 for i in blk.instructions]
        # cumulative sem counts in schedule order -> producer position
        sem_hist = {}   # sem id -> list of (position, cumulative_value)
        for pos, inst in enumerate(insts):
            si = inst.sync_info
            if si is None:
                continue
            for u in si.on_update:
                hist = sem_hist.setdefault(u.id, [])
                prev = hist[-1][1] if hist else 0
                hist.append((pos, prev + (u.update_value or 1)))

        def producer_pos(w):
            for pos, cum in sem_hist.get(w.id, ()):
                if cum >= w.wait_value:
                    return pos
            return None  # produced outside this block (host/runtime)

        def exec_unit(inst):
            """Sequential dispatch domain: the issuing engine sequencer.
            DMACopy waits are polled by the issuing sequencer (SP/ACT)
            before the descriptor is pushed, so they move within that
            engine's stream like any other instruction's waits."""
            return str(getattr(inst, "engine", None))

        # which execution units increment each semaphore.  DMA-completion
        # semaphores (DMAHW*/DMASW*) increment asynchronously at transfer
        # completion, NOT at dispatch — never treat them as same-engine.
        sem_engines = {}
        for pos, inst in enumerate(insts):
            si = inst.sync_info
            if si is None:
                continue
            for u in si.on_update:
                if u.ant_name.startswith(("DMAHW", "DMASW")):
                    sem_engines.setdefault(u.id, set()).add("ASYNC_DMA")
                else:
                    sem_engines.setdefault(u.id, set()).add(exec_unit(inst))

        n_waits = [len(i.sync_info.on_wait) if i.sync_info else 0
                   for i in insts]
        # positions of instructions per execution unit, in order
        eng_of = [exec_unit(i) for i in insts]
        # per-engine observed semaphore clock: once an engine's stream has
        # waited for (sem >= v), every later instruction on that stream
        # observes it — later waits with value <= v are redundant.
        obs = {}

        def observed(eng, w):
            return obs.get((eng, w.id), -1) >= w.wait_value

        def observe(eng, w):
            key = (eng, w.id)
            if obs.get(key, -1) < w.wait_value:
                obs[key] = w.wait_value

        for pos, inst in enumerate(insts):
            eng = eng_of[pos]
            if inst.opcode in SKIP_OPCODES or \
                    not eng.startswith("EngineType."):
                if inst.sync_info:
                    for w in inst.sync_info.on_wait:
                        observe(eng, w)
                continue
            si = inst.sync_info
            if si is None:
                continue
            waits = list(si.on_wait)
            if waits:
                # drop waits already covered by this engine's stream
                waits = [w for w in waits if not observed(eng, w)]
                # Engines retire instructions strictly in order (PE MMs are
                # pc-monotone in start AND end even across array tiles), so
                # a wait on a semaphore only ever incremented synchronously
                # by THIS engine's earlier instructions is trivially
                # satisfied: drop.  (Async DMA-completion sems excluded.)
                waits = [w for w in waits
                         if sem_engines.get(w.id) != {eng}]
            if len(waits) > 1:
                # keep one wait in place, move the rest to earlier free
                # slots on the same engine stream (after each wait's
                # producer, so the event order stays acyclic).  Prefer
                # keeping the latest-produced wait; fall back to other
                # keep choices if the excess can't be placed.
                waits.sort(key=lambda w: producer_pos(w) or len(insts))

                def try_place(keep_idx):
                    placement, used = [], set()
                    for wi, w in enumerate(waits):
                        if wi == keep_idx:
                            continue
                        pp = producer_pos(w)
                        if pp is None:
                            return None
                        tgt = None
                        for q in range(pos - 1, pp, -1):
                            if eng_of[q] == eng and n_waits[q] == 0 and \
                                    q not in used and \
                                    insts[q].opcode not in SKIP_OPCODES:
                                tgt = q
                                break
                        if tgt is None:
                            return None
                        used.add(tgt)
                        placement.append((w, tgt))
                    return placement

                placement = None
                for keep_idx in range(len(waits) - 1, -1, -1):
                    placement = try_place(keep_idx)
                    if placement is not None:
                        keep = waits[keep_idx]
                        break
                assert placement is not None, \
                    f"{inst.name}: cannot place excess waits " \
                    f"{[(w.ant_name, w.wait_value) for w in waits]}"
                for w, tgt in placement:
                    ti = insts[tgt]
                    tsi = ti.sync_info
                    ti.sync_info = bass_rust.SyncInfo(
                        on_wait=[w],
                        on_update=list(tsi.on_update)
                        if tsi is not None else [],
                    )
                    n_waits[tgt] = 1
                    observe(eng, w)
                waits = [keep]
            si.on_wait = waits
            inst.sync_info = si
            n_waits[pos] = len(waits)
            for w in waits:
                observe(eng, w)


def _prep_inputs(inputs, attention_mask, Wq, bq, Wk, bk, Wv, bv):
    """Host-side shard + layout prep.  Masked-out keys (exactly-0 softmax
    weight in the reference: exp(-10000-ish) underflows) are compacted away
    from the K/V sequence axis; pad positions carry the -10000 bias.
    Returns (per-core input maps, nk, skv)."""
    bf16 = ml_dtypes.bfloat16
    scale = 1.0 / np.sqrt(np.float32(DH))
    masks = np.asarray(attention_mask)
    has_bias = any(
        np.any(np.asarray(bias, np.float32) != 0) for bias in (bq, bk, bv))
    nk = 9 if has_bias else 8
    kpad = nk * 128
    counts = [int(masks[b].sum()) for b in range(B)]
    skv = max(1280, ((max(counts) + 127) // 128) * 128)

    in_maps = []
    xcache = {}
    for c in range(NCORES):
        b, hg = c // 2, c % 2
        if b not in xcache:
            xtf = np.asarray(inputs[b], dtype=np.float32).T  # [D, S]
            xt = np.zeros((kpad, S), dtype=bf16)
            xt[0:D, :] = xtf.astype(bf16)
            idx = np.nonzero(masks[b])[0]
            cnt = len(idx)
            xkv = np.zeros((kpad, skv), dtype=bf16)
            xkv[0:D, 0:cnt] = xtf[:, idx].astype(bf16)
            if has_bias:
                xt[D, :] = bf16(1.0)
                xkv[D, :] = bf16(1.0)
            mz = np.zeros((128, skv), dtype=bf16)
            mz[0, cnt:] = bf16(-10000.0)
            mz[64, cnt:] = bf16(-10000.0)
            xcache[b] = (xt, xkv, mz)
        xt, xkv, mz = xcache[b]
        cols = slice(hg * DCOL, (hg + 1) * DCOL)

        def wpack(W, bias, s=np.float32(1.0)):
            w = np.zeros((kpad, DCOL), dtype=bf16)
            w[0:D, :] = (np.asarray(W, np.float32)[:, cols] * s).astype(bf16)
            if has_bias:
                w[D, :] = (np.asarray(bias, np.float32)[cols] * s
                           ).astype(bf16)
            return w

        in_maps.append({
            "xt": xt,
            "xkv": xkv,
            "wq": wpack(Wq, bq),
            "wk": wpack(Wk, bk, scale),
            "wv": wpack(Wv, bv),
            "mz": mz,
        })
    return in_maps, nk, skv


_NC_CACHE = {}


def _get_nc(nk, skv):
    key = (nk, skv)
    if key not in _NC_CACHE:
        _NC_CACHE[key] = build_nc(nk, skv)
    return _NC_CACHE[key]


def _assemble(results):
    full = np.empty((B, S, D), dtype=np.float32)
    for c in range(NCORES):
        b, hg = c // 2, c % 2
        full[b, :, hg * DCOL:(hg + 1) * DCOL] = \
            np.asarray(results[c]["out"], dtype=np.float32).T
    return full


def _ensure_ntff_hook():
    """Inject the missing antenv.axon_hooks module so trace=True works."""
    import types
    try:
        from antenv import axon_hooks  # noqa: F401
        return
    except ImportError:
        pass
    import antenv
    mod = types.ModuleType("antenv.axon_hooks")
    mod._hook = None

    def set_axon_ntff_profile_hook(h):
        mod._hook = h

    def get_axon_ntff_profile_hook():
        return mod._hook

    mod.set_axon_ntff_profile_hook = set_axon_ntff_profile_hook
    mod.get_axon_ntff_profile_hook = get_axon_ntff_profile_hook
    sys.modules["antenv.axon_hooks"] = mod
    antenv.axon_hooks = mod
    from trn_agent_boot.trn_boot import _ntff_profile_via_ctypes
    mod.set_axon_ntff_profile_hook(
        _ntff_profile_via_ctypes("/opt/axon/libaxon_pjrt.so"))


def run(trace=False, **inputs):
    """Run on hardware; returns (output, BassKernelResults)."""
    from concourse.bass_utils import run_bass_kernel_spmd
    if trace:
        _ensure_ntff_hook()
    in_maps, nk, skv = _prep_inputs(**inputs)
    nc = _get_nc(nk, skv)
    res = run_bass_kernel_spmd(
        nc, in_maps, core_ids=list(range(NCORES)), trace=trace)
    return _assemble(res.results), res


def kernel(**inputs):
    out, _ = run(trace=False, **inputs)
    return out


# revision 84
# speedup vs baseline: 1.1368x; 1.1368x over previous
"""Multi-head attention layer on 8 Trainium2 NeuronCores.

Problem: B=4, S=2048, D=1024, H=16 heads (DH=64), int mask over keys.
Sharding: core c -> batch b=c//2, head-group hg=c%2 (8 heads each).
Each core computes its heads' full S x S attention independently; no
collectives.  Host-side prep folds everything expensive into the input
layout:

  - X is passed TRANSPOSED ([D, S], d on partitions) so Q^T/K^T come out
    of the projection matmuls directly in the [d, S] layout the attention
    matmuls consume (zero on-device transposes).
  - A ones-row is appended to X^T and the bias row to W so biases ride the
    projection contraction; padded with zeros to a whole 128-row K-tile so
    every projection matmul stays in the PE's 128x128 tiling mode.
  - Wk is pre-scaled by 1/sqrt(DH) on the host.
  - The additive key mask is passed as a [128, S] tensor (rows 0 and 64
    hold -10000*(1-mask), the rest zeros) and added to the scores inside
    the PE accumulation group as a zero-padded K=64 matmul, so the exp on
    ScalarE needs no per-partition bias and can span two PSUM banks.
  - V gets a ones-column (65th output row of the PV matmul) so the softmax
    denominator accumulates for free alongside the numerator.

Attention runs entirely in the PE's 64-row tiling mode: head pairs are
packed onto array halves T0 (SBUF partitions 0-63) and T8 (64-127) and run
concurrently; the K=128 PV contraction is split into T0/T8 halves that
accumulate in separate PSUM banks and are summed on the DVE at the end.
"""

import os
import sys

import numpy as np
import ml_dtypes

for _p in ("/opt/trn_rl_repo", "/opt/pypackages"):
    if os.path.isdir(_p) and _p not in sys.path:
        sys.path.append(_p)

import concourse.bass as bass
import concourse.mybir as mybir
import concourse.tile as tile
from concourse.tile import add_dep_helper
from contextlib import ExitStack

BF16 = mybir.dt.bfloat16
F32 = mybir.dt.float32

B, S, D, H, DH = 4, 2048, 1024, 16, 64
NCORES = 8
DCOL = 512          # head-group width (8 heads x 64)
NM = 4              # 128-wide dcol tiles of the head group
NQC = 4             # 512-wide query chunks
EXPFN = mybir.ActivationFunctionType.Exp


def _chunks(total, size=512):
    out, o = [], 0
    while o < total:
        c = min(size, total - o)
        out.append(slice(o, o + c))
        o += c
    return out


def build_nc(nk: int, skv: int) -> bass.Bass:
    """nk: K-tiles over the hidden dim (8, or 9 with a bias row).
    skv: compacted+padded key/value sequence length (multiple of 128).
    Masked-out keys are compacted away on the host (their exp() is an
    exact 0 in the reference); pad keys carry a -10000 mask bias."""
    NK = nk
    KPAD = NK * 128
    NJ = skv // 128     # key tiles for scores / PV / V-projection
    nc = bass.Bass()
    xt_d = nc.declare_dram_parameter("xt", [KPAD, S], BF16, isOutput=False)
    xkv_d = nc.declare_dram_parameter("xkv", [KPAD, skv], BF16,
                                      isOutput=False)
    wq_d = nc.declare_dram_parameter("wq", [KPAD, DCOL], BF16, isOutput=False)
    wk_d = nc.declare_dram_parameter("wk", [KPAD, DCOL], BF16, isOutput=False)
    wv_d = nc.declare_dram_parameter("wv", [KPAD, DCOL], BF16, isOutput=False)
    mz_d = nc.declare_dram_parameter("mz", [128, skv], BF16, isOutput=False)
    out_d = nc.declare_dram_parameter("out", [DCOL, S], F32, isOutput=True)

    with tile.TileContext(nc) as tc, ExitStack() as ctx:
        const = ctx.enter_context(tc.tile_pool(name="const", bufs=1))
        spool = ctx.enter_context(tc.tile_pool(name="sc", bufs=2, space="PSUM"))
        pvpool = ctx.enter_context(tc.tile_pool(name="pv", bufs=1, space="PSUM"))
        expool = ctx.enter_context(tc.tile_pool(name="ex", bufs=10))
        comb = ctx.enter_context(tc.tile_pool(name="comb", bufs=4))
        outp = ctx.enter_context(tc.tile_pool(name="outp", bufs=5))

        # ---------------- persistent SBUF tensors ----------------
        xt = [const.tile([128, S], BF16, tag=f"xt{k}", name=f"xt{k}")
              for k in range(NK)]
        xkv = [const.tile([128, skv], BF16, tag=f"xkv{k}", name=f"xkv{k}")
               for k in range(NK)]
        wq = [const.tile([128, DCOL], BF16, tag=f"wq{k}", name=f"wq{k}")
              for k in range(NK)]
        wk = [const.tile([128, DCOL], BF16, tag=f"wk{k}", name=f"wk{k}")
              for k in range(NK)]
        wv = [const.tile([128, DCOL], BF16, tag=f"wv{k}", name=f"wv{k}")
              for k in range(NK)]
        mz = const.tile([128, skv], BF16, tag="mz")
        ones = const.tile([128, DCOL], BF16, tag="ones")
        qt = const.tile([128, NM, S], BF16, tag="qt")    # Q^T  [dcol, S]
        kt = const.tile([128, NM, skv], BF16, tag="kt")  # K^T (pre-scaled)
        # V (cols 0-63) + 64 ones-columns (64-127) per (key tile, head): the
        # PV matmul then emits the numerator on partitions 0-63 AND the
        # softmax denominator replicated across partitions 64-127 — the
        # partition broadcast of 1/den is never needed.
        vo = const.tile([128, NJ, 8, 128], BF16, tag="vo")

        for k in range(NK):
            nc.sync.dma_start(out=xkv[k],
                              in_=xkv_d[k * 128:(k + 1) * 128, :])
            nc.sync.dma_start(out=wk[k], in_=wk_d[k * 128:(k + 1) * 128, :])
            nc.sync.dma_start(out=wv[k], in_=wv_d[k * 128:(k + 1) * 128, :])
        nc.sync.dma_start(out=mz, in_=mz_d[:, :])
        for k in range(NK):
            nc.sync.dma_start(out=xt[k], in_=xt_d[k * 128:(k + 1) * 128, :])
            nc.sync.dma_start(out=wq[k], in_=wq_d[k * 128:(k + 1) * 128, :])
        nc.gpsimd.memset(ones, 1.0)
        ms_pool = nc.gpsimd.memset(vo, 1.0)  # ones cols; V copies fill 0-63
        # DVE pre-touch: observe the gpsimd memset tick once, so the per-tile
        # V copies don't each need a second (Pool) sync wait — the DVE ISA
        # struct has a single wait slot.
        nc.vector.memset(vo[0:1, 0, 0, 64:65], 1.0)

        # ---------------- QKV projections (128x128 PE mode) ----------------
        # K^T: out[m, s] = sum_k W[k, m] * Xkv^T[k, s]
        def emit_proj(wtiles, dest, src, csl):
            for m in range(NM):
                ps = spool.tile([128, 1024], F32, tag="sc", name=f"pj{m}")
                pslice = ps[:, 0:csl.stop - csl.start]
                for k in range(NK):
                    nc.tensor.matmul(
                        pslice,
                        lhsT=wtiles[k][:, m * 128:(m + 1) * 128],
                        rhs=src[k][:, csl],
                        start=(k == 0),
                        stop=(k == NK - 1),
                    )
                nc.vector.tensor_copy(dest[:, m, csl], pslice)

        for csl in _chunks(skv):
            emit_proj(wk, kt, xkv, csl)
        # V: out[s, c] = sum_k Xkv^T[k, s] * Wv[k, c], strided into vo
        for st in range(NJ):
            ps = spool.tile([128, 1024], F32, tag="sc")
            pslice = ps[:, 0:512]
            for k in range(NK):
                nc.tensor.matmul(
                    pslice,
                    lhsT=xkv[k][:, st * 128:(st + 1) * 128],
                    rhs=wv[k],
                    start=(k == 0),
                    stop=(k == NK - 1),
                )
            nc.vector.tensor_copy(
                vo[:, st, :, 0:64],
                pslice.rearrange("p (h d) -> p h d", h=8),
            )
        # Q^T last: its input DMAs overlap the K/V compute
        for csl in _chunks(S):
            emit_proj(wq, qt, xt, csl)

        # ------- attention: software-pipelined global instruction stream --
        # Engines execute their instruction streams strictly in order, so a
        # single instruction parked on an unmet semaphore stalls everything
        # traced after it on that engine.  To keep the ScalarE exp stream
        # (the throughput limit) dense, the trace is emitted as one global
        # pipeline: scores/exp for tick g, PV matmuls for tick g-LAG, and
        # each iteration's softmax-normalization tail staggered a few ticks
        # after its last PV so all its waits are satisfied on arrival.
        LAG = 8
        NIT = NM * NQC          # 16 (pair, qchunk) iterations
        NG = NIT * NJ           # 256 scores/exp ticks
        tail_deps = []
        lasts = {}
        ex_ring = {}            # tick -> exp tile
        pv_of = {}              # it -> [pva0, pva1, pvb0, pvb1]
        cstate = {}             # (it, hh) -> dict with combine intermediates
        pending = {}            # tick -> list of closures
        exp_of_it = {}          # it -> first exp instruction (NOP anchors)

        def emit_scores(g):
            it, j = divmod(g, NJ)
            p, q = divmod(it, NQC)
            qsl = slice(q * 512, (q + 1) * 512)
            jsl = slice(j * 128, (j + 1) * 128)
            ps = spool.tile([128, 1024], F32, tag="sc", name=f"ps{g}")
            # scores^T + mask, head A on array half T0 (partitions 0-63)
            nc.tensor.matmul(
                ps[:, 0:512], lhsT=kt[0:64, p, jsl],
                rhs=qt[0:64, p, qsl], start=True, stop=False)
            nc.tensor.matmul(
                ps[:, 0:512], lhsT=mz[0:64, jsl],
                rhs=ones[0:64, :], start=False, stop=True)
            # head B on T8 (partitions 64-127)
            nc.tensor.matmul(
                ps[:, 512:1024], lhsT=kt[64:128, p, jsl],
                rhs=qt[64:128, p, qsl], start=True, stop=False)
            nc.tensor.matmul(
                ps[:, 512:1024], lhsT=mz[64:128, jsl],
                rhs=ones[64:128, :], start=False, stop=True)
            # exp over both heads' scores in one ScalarE pass
            ex = expool.tile([128, 1024], BF16, tag="ex", name=f"ex{g}")
            lasts["exp"] = nc.scalar.activation(ex, ps, EXPFN)
            ex_ring[g] = ex

        def emit_pv(g):
            it, j = divmod(g, NJ)
            p, q = divmod(it, NQC)
            if j == 0:
                pv_of[it] = [
                    pvpool.tile([128, 512], F32, tag=t, name=f"{t}_{it}")
                    for t in ("pva0", "pva1", "pvb0", "pvb1")]
            pva0, pva1, pvb0, pvb1 = pv_of[it]
            ex = ex_ring.pop(g)
            kw = dict(start=(j == 0), stop=(j == NJ - 1))
            nc.tensor.matmul(pva0, lhsT=vo[0:64, j, 2 * p, :],
                             rhs=ex[0:64, 0:512], **kw)
            nc.tensor.matmul(pva1, lhsT=vo[64:128, j, 2 * p, :],
                             rhs=ex[64:128, 0:512], **kw)
            nc.tensor.matmul(pvb0, lhsT=vo[0:64, j, 2 * p + 1, :],
                             rhs=ex[0:64, 512:1024], **kw)
            nc.tensor.matmul(pvb1, lhsT=vo[64:128, j, 2 * p + 1, :],
                             rhs=ex[64:128, 512:1024], **kw)
            if j == NJ - 1:
                # Stage the tail so every PSUM-slot-freeing copy/add runs
                # BEFORE the (slow, 3.3us) reciprocals: the next iteration's
                # PV matmuls wait on these slot releases, and a reciprocal
                # queued in between would stall the whole PE stream.
                pending.setdefault(g + 1, []).append(
                    lambda it=it: (combine1(it, 0), combine1(it, 1)))
                for hh in (0, 1):
                    pending.setdefault(g + 2 + hh, []).append(
                        lambda it=it, hh=hh: combine2(it, hh))
                    pending.setdefault(g + 4 + hh, []).append(
                        lambda it=it, hh=hh: combine3(it, hh))

        def combine1(it, hh):
            """Merge the two PV half-sums (frees both pv PSUM banks)."""
            pv0, pv1 = pv_of[it][2 * hh], pv_of[it][2 * hh + 1]
            s = cstate[(it, hh)] = {}
            tmp = comb.tile([128, 512], F32, tag="tmp", name=f"tmp{it}_{hh}")
            nc.vector.tensor_copy(tmp, pv1)
            ssum = comb.tile([128, 512], F32, tag="ssum",
                             name=f"ssum{it}_{hh}")
            s["add"] = nc.vector.tensor_add(ssum, pv0, tmp)
            s["ssum"] = ssum

        def combine2(it, hh):
            """1/den computed IN PLACE at base partition 64 (no cross-base),
            then DMA-shifted down to partitions 0-63 while the next head's
            reciprocal keeps the DVE busy — the transfer never stalls DVE."""
            s = cstate[(it, hh)]
            rect = comb.tile([128, 512], F32, tag="rect",
                             name=f"rect{it}_{hh}")
            rc_i = nc.vector.reciprocal(rect[64:128, :], s["ssum"][64:128, :])
            rec = comb.tile([64, 512], F32, tag="rec", name=f"rec{it}_{hh}")
            # SP NOP dep'd on the DMA's data producer absorbs the DVE wait
            # into the SP observed clock (1-wait DMA ISA struct)
            nop_i = nc.sync.nop(nofuse=True, hint=f"dshw{it}_{hh}")
            add_dep_helper(nop_i.ins, rc_i.ins, reason="dsh wait carry")
            s["dma"] = nc.sync.dma_start(out=rec, in_=rect[64:128, :])
            s["rec"] = rec

        def combine3(it, hh):
            """Final multiply on the otherwise-idle GpSimd engine (all
            operands are SBUF) — keeps the rec-DMA completion wait and the
            multiply itself off the busy DVE stream."""
            p, q = divmod(it, NQC)
            s = cstate.pop((it, hh))
            ot = outp.tile([64, 512], F32, tag="ot", name=f"ot{it}_{hh}")
            # absorb the rec-DMA and ssum-producer waits into the GpSimd
            # observed clock (1-wait engine ISA structs)
            scr = comb.tile([1, 1], F32, tag="scr", name=f"scr{it}_{hh}")
            m1 = nc.gpsimd.memset(scr, 0.0)
            add_dep_helper(m1.ins, s["dma"].ins, reason="rec wait carry")
            m2 = nc.gpsimd.memset(scr, 0.0)
            add_dep_helper(m2.ins, s["add"].ins, reason="ssum wait carry")
            nc.gpsimd.memset(ot[0:1, 0:1], 0.0)
            lasts["mul"] = nc.gpsimd.tensor_mul(
                ot, s["ssum"][0:64, :], s["rec"])
            nop_i = nc.sync.nop(nofuse=True, hint=f"stw{it}_{hh}")
            add_dep_helper(nop_i.ins, lasts["mul"].ins,
                           reason="store wait carry")
            row0 = p * 128 + hh * 64
            st_i = nc.sync.dma_start(
                out=out_d[row0:row0 + 64, q * 512:(q + 1) * 512], in_=ot)
            tail_deps.append(st_i)

        for g in range(NG + LAG + 8):
            for fn in pending.pop(g, ()):
                fn()
            if g < NG:
                emit_scores(g)
                if g % NJ == 0:
                    exp_of_it[g // NJ] = lasts["exp"]
            if g % 5 == 0:
                # Spread zero-wait SP slots through the stream for the
                # wait legalizer.  The dep is only for PLACEMENT: use a
                # long-completed instruction (two iterations back) so
                # the NOP's wait never stalls the SP stream — a wait on
                # the current exp would hold up every store/DMA queued
                # behind it (convoy through the ot-slot WAR).
                anchor = exp_of_it.get(min(g // NJ, NIT - 1) - 1, ms_pool)
                for k in range(8):
                    nop_i = nc.sync.nop(nofuse=True, hint=f"pad{g}_{k}")
                    add_dep_helper(nop_i.ins, anchor.ins,
                                   reason="legalizer slot padding")
            if LAG <= g < NG + LAG:
                emit_pv(g - LAG)
        for g in sorted(pending):
            for fn in pending[g]:
                fn()
        pending.clear()
        # Trailing SP no-ops, each depending on one late instruction: the
        # SP sequencer then observes every proc's final semaphore tick
        # before the kernel-tail Drain, whose ISA struct takes only a
        # single sync wait, so Tile elides the rest.
        last_store = tail_deps[-1]
        tail_deps += [lasts["exp"], lasts["mul"], ms_pool]
        for d in tail_deps:
            nop_i = nc.sync.nop(nofuse=True, hint="tailpad")
            add_dep_helper(nop_i.ins, d.ins,
                           reason="spread tail drain waits")
        for _ in range(10):  # zero-wait late slots for the legalizer
            nop_i = nc.sync.nop(nofuse=True, hint="tailpad2")
            add_dep_helper(nop_i.ins, last_store.ins,
                           reason="late zero-wait slot")
    _spread_matmul_waits(nc)
    return nc


def _spread_matmul_waits(nc):
    """The walrus in this container accepts only ONE sync-wait command per
    compute-engine ISA struct (Matmult/Activation/TensorCopy/...), but the
    Tile scheduler sometimes attaches two.  Fix: move excess waits onto an
    earlier instruction of the same engine (which executes first, so the
    ordering the wait enforces is preserved).

    Safety: a wait (sem, v) may move to predecessor p only if the
    instruction whose update makes sem reach v is scheduled BEFORE p.
    That keeps every wait's producer strictly earlier in the schedule, so
    the event order stays acyclic (no introduced deadlocks)."""
    import bass_rust

    SKIP_OPCODES = {"EventSemaphore"}
    if True:
        insts = [i for blk in nc.m.functions[0].blocks
                 for i in blk.instructions]
        # cumulative sem counts in schedule order -> producer position
        sem_hist = {}   # sem id -> list of (position, cumulative_value)
        for pos, inst in enumerate(insts):
            si = inst.sync_info
            if si is None:
                continue
            for u in si.on_update:
                hist = sem_hist.setdefault(u.id, [])
                prev = hist[-1][1] if hist else 0
                hist.append((pos, prev + (u.update_value or 1)))

        def producer_pos(w):
            for pos, cum in sem_hist.get(w.id, ()):
                if cum >= w.wait_value:
                    return pos
            return None  # produced outside this block (host/runtime)

        def exec_unit(inst):
            """Sequential dispatch domain: the issuing engine sequencer.
            DMACopy waits are polled by the issuing sequencer (SP/ACT)
            before the descriptor is pushed, so they move within that
            engine's stream like any other instruction's waits."""
            return str(getattr(inst, "engine", None))

        # which execution units increment each semaphore.  DMA-completion
        # semaphores (DMAHW*/DMASW*) increment asynchronously at transfer
        # completion, NOT at dispatch — never treat them as same-engine.
        sem_engines = {}
        for pos, inst in enumerate(insts):
            si = inst.sync_info
            if si is None:
                continue
            for u in si.on_update:
                if u.ant_name.startswith(("DMAHW", "DMASW")):
                    sem_engines.setdefault(u.id, set()).add("ASYNC_DMA")
                else:
                    sem_engines.setdefault(u.id, set()).add(exec_unit(inst))

        n_waits = [len(i.sync_info.on_wait) if i.sync_info else 0
                   for i in insts]
        # positions of instructions per execution unit, in order
        eng_of = [exec_unit(i) for i in insts]
        # per-engine observed semaphore clock: once an engine's stream has
        # waited for (sem >= v), every later instruction on that stream
        # observes it — later waits with value <= v are redundant.
        obs = {}

        def observed(eng, w):
            return obs.get((eng, w.id), -1) >= w.wait_value

        def observe(eng, w):
            key = (eng, w.id)
            if obs.get(key, -1) < w.wait_value:
                obs[key] = w.wait_value

        for pos, inst in enumerate(insts):
            eng = eng_of[pos]
            if inst.opcode in SKIP_OPCODES or \
                    not eng.startswith("EngineType."):
                if inst.sync_info:
                    for w in inst.sync_info.on_wait:
                        observe(eng, w)
                continue
            si = inst.sync_info
            if si is None:
                continue
            waits = list(si.on_wait)
            if waits:
                # drop waits already covered by this engine's stream
                waits = [w for w in waits if not observed(eng, w)]
                # Engines retire instructions strictly in order (PE MMs are
                # pc-monotone in start AND end even across array tiles), so
                # a wait on a semaphore only ever incremented synchronously
                # by THIS engine's earlier instructions is trivially
                # satisfied: drop.  (Async DMA-completion sems excluded.)
                waits = [w for w in waits
                         if sem_engines.get(w.id) != {eng}]
            if len(waits) > 1:
                # keep one wait in place, move the rest to earlier free
                # slots on the same engine stream (after each wait's
                # producer, so the event order stays acyclic).  Prefer
                # keeping the latest-produced wait; fall back to other
                # keep choices if the excess can't be placed.
                waits.sort(key=lambda w: producer_pos(w) or len(insts))

                def try_place(keep_idx):
                    placement, used = [], set()
                    for wi, w in enumerate(waits):
                        if wi == keep_idx:
                            continue
                        pp = producer_pos(w)
                        if pp is None:
                            return None
                        tgt = None
                        for q in range(pos - 1, pp, -1):
                            if eng_of[q] == eng and n_waits[q] == 0 and \
                                    q not in used and \
                                    insts[q].opcode not in SKIP_OPCODES:
                                tgt = q
                                break
                        if tgt is None:
                            return None
                        used.add(tgt)
                        placement.append((w, tgt))
                    return placement

                placement = None
                for keep_idx in range(len(waits) - 1, -1, -1):
                    placement = try_place(keep_idx)
                    if placement is not None:
                        keep = waits[keep_idx]
                        break
                assert placement is not None, \
                    f"{inst.name}: cannot place excess waits " \
                    f"{[(w.ant_name, w.wait_value) for w in waits]}"
                for w, tgt in placement:
                    ti = insts[tgt]
                    tsi = ti.sync_info
                    ti.sync_info = bass_rust.SyncInfo(
                        on_wait=[w],
                        on_update=list(tsi.on_update)
                        if tsi is not None else [],
                    )
                    n_waits[tgt] = 1
                    observe(eng, w)
                waits = [keep]
            si.on_wait = waits
            inst.sync_info = si
            n_waits[pos] = len(waits)
            for w in waits:
                observe(eng, w)


def _prep_inputs(inputs, attention_mask, Wq, bq, Wk, bk, Wv, bv):
    """Host-side shard + layout prep.  Masked-out keys (exactly-0 softmax
    weight in the reference: exp(-10000-ish) underflows) are compacted away
    from the K/V sequence axis; pad positions carry the -10000 bias.
    Returns (per-core input maps, nk, skv)."""
    bf16 = ml_dtypes.bfloat16
    scale = 1.0 / np.sqrt(np.float32(DH))
    masks = np.asarray(attention_mask)
    has_bias = any(
        np.any(np.asarray(bias, np.float32) != 0) for bias in (bq, bk, bv))
    nk = 9 if has_bias else 8
    kpad = nk * 128
    counts = [int(masks[b].sum()) for b in range(B)]
    skv = max(1280, ((max(counts) + 127) // 128) * 128)

    in_maps = []
    xcache = {}
    for c in range(NCORES):
        b, hg = c // 2, c % 2
        if b not in xcache:
            xtf = np.asarray(inputs[b], dtype=np.float32).T  # [D, S]
            xt = np.zeros((kpad, S), dtype=bf16)
            xt[0:D, :] = xtf.astype(bf16)
            idx = np.nonzero(masks[b])[0]
            cnt = len(idx)
            xkv = np.zeros((kpad, skv), dtype=bf16)
            xkv[0:D, 0:cnt] = xtf[:, idx].astype(bf16)
            if has_bias:
                xt[D, :] = bf16(1.0)
                xkv[D, :] = bf16(1.0)
            mz = np.zeros((128, skv), dtype=bf16)
            mz[0, cnt:] = bf16(-10000.0)
            mz[64, cnt:] = bf16(-10000.0)
            xcache[b] = (xt, xkv, mz)
        xt, xkv, mz = xcache[b]
        cols = slice(hg * DCOL, (hg + 1) * DCOL)

        def wpack(W, bias, s=np.float32(1.0)):
            w = np.zeros((kpad, DCOL), dtype=bf16)
            w[0:D, :] = (np.asarray(W, np.float32)[:, cols] * s).astype(bf16)
            if has_bias:
                w[D, :] = (np.asarray(bias, np.float32)[cols] * s
                           ).astype(bf16)
            return w

        in_maps.append({
            "xt": xt,
            "xkv": xkv,
            "wq": wpack(Wq, bq),
            "wk": wpack(Wk, bk, scale),
            "wv": wpack(Wv, bv),
            "mz": mz,
        })
    return in_maps, nk, skv


_NC_CACHE = {}


def _get_nc(nk, skv):
    key = (nk, skv)
    if key not in _NC_CACHE:
        _NC_CACHE[key] = build_nc(nk, skv)
    return _NC_CACHE[key]


def _assemble(results):
    full = np.empty((B, S, D), dtype=np.float32)
    for c in range(NCORES):
        b, hg = c // 2, c % 2
        full[b, :, hg * DCOL:(hg + 1) * DCOL] = \
            np.asarray(results[c]["out"], dtype=np.float32).T
    return full


def _ensure_ntff_hook():
    """Inject the missing antenv.axon_hooks module so trace=True works."""
    import types
    try:
        from antenv import axon_hooks  # noqa: F401
        return
    except ImportError:
        pass
    import antenv
    mod = types.ModuleType("antenv.axon_hooks")
    mod._hook = None

    def set_axon_ntff_profile_hook(h):
        mod._hook = h

    def get_axon_ntff_profile_hook():
        return mod._hook

    mod.set_axon_ntff_profile_hook = set_axon_ntff_profile_hook
    mod.get_axon_ntff_profile_hook = get_axon_ntff_profile_hook
    sys.modules["antenv.axon_hooks"] = mod
    antenv.axon_hooks = mod
    from trn_agent_boot.trn_boot import _ntff_profile_via_ctypes
    mod.set_axon_ntff_profile_hook(
        _ntff_profile_via_ctypes("/opt/axon/libaxon_pjrt.so"))


def run(trace=False, **inputs):
    """Run on hardware; returns (output, BassKernelResults)."""
    from concourse.bass_utils import run_bass_kernel_spmd
    if trace:
        _ensure_ntff_hook()
    in_maps, nk, skv = _prep_inputs(**inputs)
    nc = _get_nc(nk, skv)
    res = run_bass_kernel_spmd(
        nc, in_maps, core_ids=list(range(NCORES)), trace=trace)
    return _assemble(res.results), res


def kernel(**inputs):
    out, _ = run(trace=False, **inputs)
    return out


# revision 85
# speedup vs baseline: 1.2146x; 1.0684x over previous
"""Multi-head attention layer on 8 Trainium2 NeuronCores.

Problem: B=4, S=2048, D=1024, H=16 heads (DH=64), int mask over keys.
Sharding: core c -> batch b=c//2, head-group hg=c%2 (8 heads each).
Each core computes its heads' full S x S attention independently; no
collectives.  Host-side prep folds everything expensive into the input
layout:

  - X is passed TRANSPOSED ([D, S], d on partitions) so Q^T/K^T come out
    of the projection matmuls directly in the [d, S] layout the attention
    matmuls consume (zero on-device transposes).
  - A ones-row is appended to X^T and the bias row to W so biases ride the
    projection contraction; padded with zeros to a whole 128-row K-tile so
    every projection matmul stays in the PE's 128x128 tiling mode.
  - Wk is pre-scaled by 1/sqrt(DH) on the host.
  - The additive key mask is passed as a [128, S] tensor (rows 0 and 64
    hold -10000*(1-mask), the rest zeros) and added to the scores inside
    the PE accumulation group as a zero-padded K=64 matmul, so the exp on
    ScalarE needs no per-partition bias and can span two PSUM banks.
  - V gets a ones-column (65th output row of the PV matmul) so the softmax
    denominator accumulates for free alongside the numerator.

Attention runs entirely in the PE's 64-row tiling mode: head pairs are
packed onto array halves T0 (SBUF partitions 0-63) and T8 (64-127) and run
concurrently; the K=128 PV contraction is split into T0/T8 halves that
accumulate in separate PSUM banks and are summed on the DVE at the end.
"""

import os
import sys

import numpy as np
import ml_dtypes

for _p in ("/opt/trn_rl_repo", "/opt/pypackages"):
    if os.path.isdir(_p) and _p not in sys.path:
        sys.path.append(_p)

import concourse.bass as bass
import concourse.mybir as mybir
import concourse.tile as tile
from concourse.tile import add_dep_helper
from contextlib import ExitStack

BF16 = mybir.dt.bfloat16
F32 = mybir.dt.float32

B, S, D, H, DH = 4, 2048, 1024, 16, 64
NCORES = 8
DCOL = 512          # head-group width (8 heads x 64)
NM = 4              # 128-wide dcol tiles of the head group
NQC = 4             # 512-wide query chunks
EXPFN = mybir.ActivationFunctionType.Exp


def _chunks(total, size=512):
    out, o = [], 0
    while o < total:
        c = min(size, total - o)
        out.append(slice(o, o + c))
        o += c
    return out


def build_nc(nk: int, skv: int) -> bass.Bass:
    """nk: K-tiles over the hidden dim (8, or 9 with a bias row).
    skv: compacted+padded key/value sequence length (multiple of 128).
    Masked-out keys are compacted away on the host (their exp() is an
    exact 0 in the reference); pad keys carry a -10000 mask bias."""
    NK = nk
    KPAD = NK * 128
    NJ = skv // 128     # key tiles for scores / PV / V-projection
    nc = bass.Bass()
    xt_d = nc.declare_dram_parameter("xt", [KPAD, S], BF16, isOutput=False)
    xkv_d = nc.declare_dram_parameter("xkv", [KPAD, skv], BF16,
                                      isOutput=False)
    wq_d = nc.declare_dram_parameter("wq", [KPAD, DCOL], BF16, isOutput=False)
    wk_d = nc.declare_dram_parameter("wk", [KPAD, DCOL], BF16, isOutput=False)
    wv_d = nc.declare_dram_parameter("wv", [KPAD, DCOL], BF16, isOutput=False)
    mz_d = nc.declare_dram_parameter("mz", [128, skv], BF16, isOutput=False)
    out_d = nc.declare_dram_parameter("out", [DCOL, S], F32, isOutput=True)

    with tile.TileContext(nc) as tc, ExitStack() as ctx:
        const = ctx.enter_context(tc.tile_pool(name="const", bufs=1))
        spool = ctx.enter_context(tc.tile_pool(name="sc", bufs=2, space="PSUM"))
        pvpool = ctx.enter_context(tc.tile_pool(name="pv", bufs=1, space="PSUM"))
        expool = ctx.enter_context(tc.tile_pool(name="ex", bufs=10))
        comb = ctx.enter_context(tc.tile_pool(name="comb", bufs=4))
        outp = ctx.enter_context(tc.tile_pool(name="outp", bufs=5))

        # ---------------- persistent SBUF tensors ----------------
        xt = [const.tile([128, S], BF16, tag=f"xt{k}", name=f"xt{k}")
              for k in range(NK)]
        xkv = [const.tile([128, skv], BF16, tag=f"xkv{k}", name=f"xkv{k}")
               for k in range(NK)]
        wq = [const.tile([128, DCOL], BF16, tag=f"wq{k}", name=f"wq{k}")
              for k in range(NK)]
        wk = [const.tile([128, DCOL], BF16, tag=f"wk{k}", name=f"wk{k}")
              for k in range(NK)]
        wv = [const.tile([128, DCOL], BF16, tag=f"wv{k}", name=f"wv{k}")
              for k in range(NK)]
        mz = const.tile([128, skv], BF16, tag="mz")
        ones = const.tile([128, DCOL], BF16, tag="ones")
        qt = const.tile([128, NM, S], BF16, tag="qt")    # Q^T  [dcol, S]
        kt = const.tile([128, NM, skv], BF16, tag="kt")  # K^T (pre-scaled)
        # V (cols 0-63) + 64 ones-columns (64-127) per (key tile, head): the
        # PV matmul then emits the numerator on partitions 0-63 AND the
        # softmax denominator replicated across partitions 64-127 — the
        # partition broadcast of 1/den is never needed.
        vo = const.tile([128, NJ, 8, 128], BF16, tag="vo")

        for k in range(NK):
            nc.sync.dma_start(out=xkv[k],
                              in_=xkv_d[k * 128:(k + 1) * 128, :])
            nc.sync.dma_start(out=wk[k], in_=wk_d[k * 128:(k + 1) * 128, :])
            nc.sync.dma_start(out=wv[k], in_=wv_d[k * 128:(k + 1) * 128, :])
        nc.sync.dma_start(out=mz, in_=mz_d[:, :])
        for k in range(NK):
            nc.sync.dma_start(out=xt[k], in_=xt_d[k * 128:(k + 1) * 128, :])
            nc.sync.dma_start(out=wq[k], in_=wq_d[k * 128:(k + 1) * 128, :])
        nc.gpsimd.memset(ones, 1.0)
        ms_pool = nc.gpsimd.memset(vo, 1.0)  # ones cols; V copies fill 0-63
        # DVE pre-touch: observe the gpsimd memset tick once, so the per-tile
        # V copies don't each need a second (Pool) sync wait — the DVE ISA
        # struct has a single wait slot.
        nc.vector.memset(vo[0:1, 0, 0, 64:65], 1.0)

        # ---------------- QKV projections (128x128 PE mode) ----------------
        # K^T: out[m, s] = sum_k W[k, m] * Xkv^T[k, s]
        def emit_proj(wtiles, dest, src, csl):
            for m in range(NM):
                ps = spool.tile([128, 1024], F32, tag="sc", name=f"pj{m}")
                pslice = ps[:, 0:csl.stop - csl.start]
                for k in range(NK):
                    nc.tensor.matmul(
                        pslice,
                        lhsT=wtiles[k][:, m * 128:(m + 1) * 128],
                        rhs=src[k][:, csl],
                        start=(k == 0),
                        stop=(k == NK - 1),
                    )
                nc.vector.tensor_copy(dest[:, m, csl], pslice)

        for csl in _chunks(skv):
            emit_proj(wk, kt, xkv, csl)
        # V: out[s, c] = sum_k Xkv^T[k, s] * Wv[k, c], strided into vo
        for st in range(NJ):
            ps = spool.tile([128, 1024], F32, tag="sc")
            pslice = ps[:, 0:512]
            for k in range(NK):
                nc.tensor.matmul(
                    pslice,
                    lhsT=xkv[k][:, st * 128:(st + 1) * 128],
                    rhs=wv[k],
                    start=(k == 0),
                    stop=(k == NK - 1),
                )
            nc.vector.tensor_copy(
                vo[:, st, :, 0:64],
                pslice.rearrange("p (h d) -> p h d", h=8),
            )
        # Q^T last: its input DMAs overlap the K/V compute
        for csl in _chunks(S):
            emit_proj(wq, qt, xt, csl)

        # ------- attention: software-pipelined global instruction stream --
        # Engines execute their instruction streams strictly in order, so a
        # single instruction parked on an unmet semaphore stalls everything
        # traced after it on that engine.  To keep the ScalarE exp stream
        # (the throughput limit) dense, the trace is emitted as one global
        # pipeline: scores/exp for tick g, PV matmuls for tick g-LAG, and
        # each iteration's softmax-normalization tail staggered a few ticks
        # after its last PV so all its waits are satisfied on arrival.
        LAG = 8
        NIT = NM * NQC          # 16 (pair, qchunk) iterations
        NG = NIT * NJ           # 256 scores/exp ticks
        tail_deps = []
        lasts = {}
        ex_ring = {}            # tick -> exp tile
        pv_of = {}              # it -> [pva0, pva1, pvb0, pvb1]
        cstate = {}             # (it, hh) -> dict with combine intermediates
        pending = {}            # tick -> list of closures
        exp_of_it = {}          # it -> first exp instruction (NOP anchors)

        def emit_scores(g):
            it, j = divmod(g, NJ)
            p, q = divmod(it, NQC)
            qsl = slice(q * 512, (q + 1) * 512)
            jsl = slice(j * 128, (j + 1) * 128)
            ps = spool.tile([128, 1024], F32, tag="sc", name=f"ps{g}")
            # scores^T + mask, head A on array half T0 (partitions 0-63)
            nc.tensor.matmul(
                ps[:, 0:512], lhsT=kt[0:64, p, jsl],
                rhs=qt[0:64, p, qsl], start=True, stop=False)
            nc.tensor.matmul(
                ps[:, 0:512], lhsT=mz[0:64, jsl],
                rhs=ones[0:64, :], start=False, stop=True)
            # head B on T8 (partitions 64-127)
            nc.tensor.matmul(
                ps[:, 512:1024], lhsT=kt[64:128, p, jsl],
                rhs=qt[64:128, p, qsl], start=True, stop=False)
            nc.tensor.matmul(
                ps[:, 512:1024], lhsT=mz[64:128, jsl],
                rhs=ones[64:128, :], start=False, stop=True)
            # exp over both heads' scores in one ScalarE pass
            ex = expool.tile([128, 1024], BF16, tag="ex", name=f"ex{g}")
            lasts["exp"] = nc.scalar.activation(ex, ps, EXPFN)
            ex_ring[g] = ex

        def emit_pv(g):
            it, j = divmod(g, NJ)
            p, q = divmod(it, NQC)
            if j == 0:
                pv_of[it] = [
                    pvpool.tile([128, 512], F32, tag=t, name=f"{t}_{it}")
                    for t in ("pva0", "pva1", "pvb0", "pvb1")]
            pva0, pva1, pvb0, pvb1 = pv_of[it]
            ex = ex_ring.pop(g)
            kw = dict(start=(j == 0), stop=(j == NJ - 1))
            nc.tensor.matmul(pva0, lhsT=vo[0:64, j, 2 * p, :],
                             rhs=ex[0:64, 0:512], **kw)
            nc.tensor.matmul(pva1, lhsT=vo[64:128, j, 2 * p, :],
                             rhs=ex[64:128, 0:512], **kw)
            nc.tensor.matmul(pvb0, lhsT=vo[0:64, j, 2 * p + 1, :],
                             rhs=ex[0:64, 512:1024], **kw)
            nc.tensor.matmul(pvb1, lhsT=vo[64:128, j, 2 * p + 1, :],
                             rhs=ex[64:128, 512:1024], **kw)
            if j == NJ - 1:
                # Stage the tail so every PSUM-slot-freeing copy/add runs
                # BEFORE the (slow, 3.3us) reciprocals: the next iteration's
                # PV matmuls wait on these slot releases, and a reciprocal
                # queued in between would stall the whole PE stream.
                pending.setdefault(g + 1, []).append(
                    lambda it=it: (combine1(it, 0), combine1(it, 1)))
                for hh in (0, 1):
                    pending.setdefault(g + 2 + hh, []).append(
                        lambda it=it, hh=hh: combine2(it, hh))
                    pending.setdefault(g + 4 + hh, []).append(
                        lambda it=it, hh=hh: combine3(it, hh))

        def combine1(it, hh):
            """Merge the two PV half-sums (frees both pv PSUM banks)."""
            pv0, pv1 = pv_of[it][2 * hh], pv_of[it][2 * hh + 1]
            s = cstate[(it, hh)] = {}
            tmp = comb.tile([128, 512], F32, tag="tmp", name=f"tmp{it}_{hh}")
            nc.vector.tensor_copy(tmp, pv1)
            ssum = comb.tile([128, 512], F32, tag="ssum",
                             name=f"ssum{it}_{hh}")
            s["add"] = nc.vector.tensor_add(ssum, pv0, tmp)
            s["ssum"] = ssum

        def combine2(it, hh):
            """1/den computed IN PLACE at base partition 64 (no cross-base),
            then DMA-shifted down to partitions 0-63 while the next head's
            reciprocal keeps the DVE busy — the transfer never stalls DVE."""
            s = cstate[(it, hh)]
            rect = comb.tile([128, 512], F32, tag="rect",
                             name=f"rect{it}_{hh}")
            rc_i = nc.vector.reciprocal(rect[64:128, :], s["ssum"][64:128, :])
            rec = comb.tile([64, 512], F32, tag="rec", name=f"rec{it}_{hh}")
            # SP NOP dep'd on the DMA's data producer absorbs the DVE wait
            # into the SP observed clock (1-wait DMA ISA struct)
            nop_i = nc.sync.nop(nofuse=True, hint=f"dshw{it}_{hh}")
            add_dep_helper(nop_i.ins, rc_i.ins, reason="dsh wait carry")
            s["dma"] = nc.sync.dma_start(out=rec, in_=rect[64:128, :])
            s["rec"] = rec

        def combine3(it, hh):
            """Final multiply on the otherwise-idle GpSimd engine (all
            operands are SBUF) — keeps the rec-DMA completion wait and the
            multiply itself off the busy DVE stream."""
            p, q = divmod(it, NQC)
            s = cstate.pop((it, hh))
            ot = outp.tile([64, 512], F32, tag="ot", name=f"ot{it}_{hh}")
            # absorb the rec-DMA and ssum-producer waits into the GpSimd
            # observed clock (1-wait engine ISA structs)
            scr = comb.tile([1, 1], F32, tag="scr", name=f"scr{it}_{hh}")
            m1 = nc.gpsimd.memset(scr, 0.0)
            add_dep_helper(m1.ins, s["dma"].ins, reason="rec wait carry")
            m2 = nc.gpsimd.memset(scr, 0.0)
            add_dep_helper(m2.ins, s["add"].ins, reason="ssum wait carry")
            nc.gpsimd.memset(ot[0:1, 0:1], 0.0)
            lasts["mul"] = nc.gpsimd.tensor_mul(
                ot, s["ssum"][0:64, :], s["rec"])
            nop_i = nc.sync.nop(nofuse=True, hint=f"stw{it}_{hh}")
            add_dep_helper(nop_i.ins, lasts["mul"].ins,
                           reason="store wait carry")
            row0 = p * 128 + hh * 64
            st_i = nc.sync.dma_start(
                out=out_d[row0:row0 + 64, q * 512:(q + 1) * 512], in_=ot)
            tail_deps.append(st_i)

        for g in range(NG + LAG + 8):
            for fn in pending.pop(g, ()):
                fn()
            if g < NG:
                emit_scores(g)
                if g % NJ == 0:
                    exp_of_it[g // NJ] = lasts["exp"]
            if g % 5 == 0:
                # Spread zero-wait SP slots through the stream for the
                # wait legalizer.  The dep is only for PLACEMENT: use a
                # long-completed instruction (two iterations back) so
                # the NOP's wait never stalls the SP stream — a wait on
                # the current exp would hold up every store/DMA queued
                # behind it (convoy through the ot-slot WAR).
                anchor = exp_of_it.get(min(g // NJ, NIT - 1) - 1, ms_pool)
                for k in range(8):
                    nop_i = nc.sync.nop(nofuse=True, hint=f"pad{g}_{k}")
                    add_dep_helper(nop_i.ins, anchor.ins,
                                   reason="legalizer slot padding")
            if LAG <= g < NG + LAG:
                emit_pv(g - LAG)
        for g in sorted(pending):
            for fn in pending[g]:
                fn()
        pending.clear()
        # Trailing SP no-ops, each depending on one late instruction: the
        # SP sequencer then observes every proc's final semaphore tick
        # before the kernel-tail Drain, whose ISA struct takes only a
        # single sync wait, so Tile elides the rest.
        last_store = tail_deps[-1]
        tail_deps += [lasts["exp"], lasts["mul"], ms_pool]
        for d in tail_deps:
            nop_i = nc.sync.nop(nofuse=True, hint="tailpad")
            add_dep_helper(nop_i.ins, d.ins,
                           reason="spread tail drain waits")
        for _ in range(10):  # zero-wait late slots for the legalizer
            nop_i = nc.sync.nop(nofuse=True, hint="tailpad2")
            add_dep_helper(nop_i.ins, last_store.ins,
                           reason="late zero-wait slot")
    _spread_matmul_waits(nc)
    return nc


def _spread_matmul_waits(nc):
    """The walrus in this container accepts only ONE sync-wait command per
    compute-engine ISA struct (Matmult/Activation/TensorCopy/...), but the
    Tile scheduler sometimes attaches two.  Fix: move excess waits onto an
    earlier instruction of the same engine (which executes first, so the
    ordering the wait enforces is preserved).

    Safety: a wait (sem, v) may move to predecessor p only if the
    instruction whose update makes sem reach v is scheduled BEFORE p.
    That keeps every wait's producer strictly earlier in the schedule, so
    the event order stays acyclic (no introduced deadlocks)."""
    import bass_rust

    SKIP_OPCODES = {"EventSemaphore"}
    if True:
        insts = [i for blk in nc.m.functions[0].blocks
                 for i in blk.instructions]
        # cumulative sem counts in schedule order -> producer position
        sem_hist = {}   # sem id -> list of (position, cumulative_value)
        for pos, inst in enumerate(insts):
            si = inst.sync_info
            if si is None:
                continue
            for u in si.on_update:
                hist = sem_hist.setdefault(u.id, [])
                prev = hist[-1][1] if hist else 0
                hist.append((pos, prev + (u.update_value or 1)))

        def producer_pos(w):
            for pos, cum in sem_hist.get(w.id, ()):
                if cum >= w.wait_value:
                    return pos
            return None  # produced outside this block (host/runtime)

        def exec_unit(inst):
            """Sequential dispatch domain: the issuing engine sequencer.
            DMACopy waits are polled by the issuing sequencer (SP/ACT)
            before the descriptor is pushed, so they move within that
            engine's stream like any other instruction's waits."""
            return str(getattr(inst, "engine", None))

        # which execution units increment each semaphore.  DMA-completion
        # semaphores (DMAHW*/DMASW*) increment asynchronously at transfer
        # completion, NOT at dispatch — never treat them as same-engine.
        sem_engines = {}
        for pos, inst in enumerate(insts):
            si = inst.sync_info
            if si is None:
                continue
            for u in si.on_update:
                if u.ant_name.startswith(("DMAHW", "DMASW")):
                    sem_engines.setdefault(u.id, set()).add("ASYNC_DMA")
                else:
                    sem_engines.setdefault(u.id, set()).add(exec_unit(inst))

        n_waits = [len(i.sync_info.on_wait) if i.sync_info else 0
                   for i in insts]
        # positions of instructions per execution unit, in order
        eng_of = [exec_unit(i) for i in insts]
        # per-engine observed semaphore clock: once an engine's stream has
        # waited for (sem >= v), every later instruction on that stream
        # observes it — later waits with value <= v are redundant.
        obs = {}

        def observed(eng, w):
            return obs.get((eng, w.id), -1) >= w.wait_value

        def observe(eng, w):
            key = (eng, w.id)
            if obs.get(key, -1) < w.wait_value:
                obs[key] = w.wait_value

        for pos, inst in enumerate(insts):
            eng = eng_of[pos]
            if inst.opcode in SKIP_OPCODES or \
                    not eng.startswith("EngineType."):
                if inst.sync_info:
                    for w in inst.sync_info.on_wait:
                        observe(eng, w)
                continue
            si = inst.sync_info
            if si is None:
                continue
            waits = list(si.on_wait)
            if waits:
                # drop waits already covered by this engine's stream
                waits = [w for w in waits if not observed(eng, w)]
                # Engines retire instructions strictly in order (PE MMs are
                # pc-monotone in start AND end even across array tiles), so
                # a wait on a semaphore only ever incremented synchronously
                # by THIS engine's earlier instructions is trivially
                # satisfied: drop.  (Async DMA-completion sems excluded.)
                waits = [w for w in waits
                         if sem_engines.get(w.id) != {eng}]
            if len(waits) > 1:
                # keep one wait in place, move the rest to earlier free
                # slots on the same engine stream (after each wait's
                # producer, so the event order stays acyclic).  Prefer
                # keeping the latest-produced wait; fall back to other
                # keep choices if the excess can't be placed.
                waits.sort(key=lambda w: producer_pos(w) or len(insts))

                def try_place(keep_idx):
                    placement, used = [], set()
                    for wi, w in enumerate(waits):
                        if wi == keep_idx:
                            continue
                        pp = producer_pos(w)
                        if pp is None:
                            return None
                        tgt = None
                        for q in range(pos - 1, pp, -1):
                            if eng_of[q] == eng and n_waits[q] == 0 and \
                                    q not in used and \
                                    insts[q].opcode not in SKIP_OPCODES:
                                tgt = q
                                break
                        if tgt is None:
                            return None
                        used.add(tgt)
                        placement.append((w, tgt))
                    return placement

                placement = None
                for keep_idx in range(len(waits) - 1, -1, -1):
                    placement = try_place(keep_idx)
                    if placement is not None:
                        keep = waits[keep_idx]
                        break
                assert placement is not None, \
                    f"{inst.name}: cannot place excess waits " \
                    f"{[(w.ant_name, w.wait_value) for w in waits]}"
                for w, tgt in placement:
                    ti = insts[tgt]
                    tsi = ti.sync_info
                    ti.sync_info = bass_rust.SyncInfo(
                        on_wait=[w],
                        on_update=list(tsi.on_update)
                        if tsi is not None else [],
                    )
                    n_waits[tgt] = 1
                    observe(eng, w)
                waits = [keep]
            si.on_wait = waits
            inst.sync_info = si
            n_waits[pos] = len(waits)
            for w in waits:
                observe(eng, w)


def _prep_inputs(inputs, attention_mask, Wq, bq, Wk, bk, Wv, bv):
    """Host-side shard + layout prep.  Masked-out keys (exactly-0 softmax
    weight in the reference: exp(-10000-ish) underflows) are compacted away
    from the K/V sequence axis; pad positions carry the -10000 bias.
    Returns (per-core input maps, nk, skv)."""
    bf16 = ml_dtypes.bfloat16
    scale = 1.0 / np.sqrt(np.float32(DH))
    masks = np.asarray(attention_mask)
    has_bias = any(
        np.any(np.asarray(bias, np.float32) != 0) for bias in (bq, bk, bv))
    nk = 9 if has_bias else 8
    kpad = nk * 128
    counts = [int(masks[b].sum()) for b in range(B)]
    skv = ((max(counts) + 127) // 128) * 128

    in_maps = []
    xcache = {}
    for c in range(NCORES):
        b, hg = c // 2, c % 2
        if b not in xcache:
            xtf = np.asarray(inputs[b], dtype=np.float32).T  # [D, S]
            xt = np.zeros((kpad, S), dtype=bf16)
            xt[0:D, :] = xtf.astype(bf16)
            idx = np.nonzero(masks[b])[0]
            cnt = len(idx)
            xkv = np.zeros((kpad, skv), dtype=bf16)
            xkv[0:D, 0:cnt] = xtf[:, idx].astype(bf16)
            if has_bias:
                xt[D, :] = bf16(1.0)
                xkv[D, :] = bf16(1.0)
            mz = np.zeros((128, skv), dtype=bf16)
            mz[0, cnt:] = bf16(-10000.0)
            mz[64, cnt:] = bf16(-10000.0)
            xcache[b] = (xt, xkv, mz)
        xt, xkv, mz = xcache[b]
        cols = slice(hg * DCOL, (hg + 1) * DCOL)

        def wpack(W, bias, s=np.float32(1.0)):
            w = np.zeros((kpad, DCOL), dtype=bf16)
            w[0:D, :] = (np.asarray(W, np.float32)[:, cols] * s).astype(bf16)
            if has_bias:
                w[D, :] = (np.asarray(bias, np.float32)[cols] * s
                           ).astype(bf16)
            return w

        in_maps.append({
            "xt": xt,
            "xkv": xkv,
            "wq": wpack(Wq, bq),
            "wk": wpack(Wk, bk, scale),
            "wv": wpack(Wv, bv),
            "mz": mz,
        })
    return in_maps, nk, skv


_NC_CACHE = {}


def _get_nc(nk, skv):
    key = (nk, skv)
    if key not in _NC_CACHE:
        _NC_CACHE[key] = build_nc(nk, skv)
    return _NC_CACHE[key]


def _assemble(results):
    full = np.empty((B, S, D), dtype=np.float32)
    for c in range(NCORES):
        b, hg = c // 2, c % 2
        full[b, :, hg * DCOL:(hg + 1) * DCOL] = \
            np.asarray(results[c]["out"], dtype=np.float32).T
    return full


def _ensure_ntff_hook():
    """Inject the missing antenv.axon_hooks module so trace=True works."""
    import types
    try:
        from antenv import axon_hooks  # noqa: F401
        return
    except ImportError:
        pass
    import antenv
    mod = types.ModuleType("antenv.axon_hooks")
    mod._hook = None

    def set_axon_ntff_profile_hook(h):
        mod._hook = h

    def get_axon_ntff_profile_hook():
        return mod._hook

    mod.set_axon_ntff_profile_hook = set_axon_ntff_profile_hook
    mod.get_axon_ntff_profile_hook = get_axon_ntff_profile_hook
    sys.modules["antenv.axon_hooks"] = mod
    antenv.axon_hooks = mod
    from trn_agent_boot.trn_boot import _ntff_profile_via_ctypes
    mod.set_axon_ntff_profile_hook(
        _ntff_profile_via_ctypes("/opt/axon/libaxon_pjrt.so"))


def run(trace=False, **inputs):
    """Run on hardware; returns (output, BassKernelResults)."""
    from concourse.bass_utils import run_bass_kernel_spmd
    if trace:
        _ensure_ntff_hook()
    in_maps, nk, skv = _prep_inputs(**inputs)
    nc = _get_nc(nk, skv)
    res = run_bass_kernel_spmd(
        nc, in_maps, core_ids=list(range(NCORES)), trace=trace)
    return _assemble(res.results), res


def kernel(**inputs):
    out, _ = run(trace=False, **inputs)
    return out
